# revision 8
# baseline (speedup 1.0000x reference)
"""GIN (3-layer) + global mean pool + MLP head on 8 Trainium2 NeuronCores.

Strategy (v2): shard nodes (dst) across 8 cores. The exchanged table holds
y = h @ W1 (W1 folded into the producing pass), bf16, FEATURE-major, resident
in SBUF [128, 50176]. Per 128-dst-node chunk, edge source columns are gathered
with a single GPSIMD indirect_copy (uint16 column indices, no DMA-descriptor
generation), transposed to edge-major on the PE, and segment-summed via
accumulating one-hot matmuls (one-hots precomputed on host, shipped bf16).
z1 = psum + y_own + b1 directly (no dense W1 matmul in pass A). Training-mode
BatchNorm stats via per-chunk reduce (sum on DVE, sumsq on Act with accum_out)
and a tiny AllReduce. Pass B: BN+ReLU, W2 matmul, inter-layer ReLU, then the
next layer's W1 applied immediately so the AllGather ships y'. Pooling via
per-chunk transpose + matmul with 1/count weights; readout MLP in fp32.
"""
import sys
sys.path.insert(0, "/opt/trn_rl_repo")
import numpy as np
import ml_dtypes

import concourse.bass as bass
import concourse.bacc as bacc
import concourse.mybir as mybir
import concourse.tile as tile
from concourse.bass_utils import run_bass_kernel_spmd
from concourse.masks import make_identity

N = 50000
E = 500000
D = 128
L = 3
G = 64
OUT = 16
EPS = 1e-5
NCORES = 8
SH = N // NCORES          # 6250 nodes per core
NCH = (SH + 127) // 128   # 49 chunks
SHP = NCH * 128           # 6272 padded shard rows
TR = NCORES * SHP         # 50176 table columns
LASTC = SH - (NCH - 1) * 128  # 106 real cols in last chunk

F32 = mybir.dt.float32
BF16 = mybir.dt.bfloat16
I16 = mybir.dt.int16
NBF = ml_dtypes.bfloat16

_cache = {}


def _build(Spairs):
    SE = [p[0] for p in Spairs]
    SO = [p[1] for p in Spairs]
    S = [a + b for a, b in Spairs]
    TOTS = sum(S)
    offs = np.concatenate([[0], np.cumsum(S)]).astype(int)
    SMAX = max(S)

    nc = bacc.Bacc(None, num_devices=NCORES)
    P = lambda n, s, dt=F32: nc.declare_dram_parameter(n, s, dt, isOutput=False)

    x0tab = P("x0tab", [128, TR], BF16)
    y0 = P("y0", [128, SHP], F32)
    idxd = P("idxd", [128, TOTS * 8], I16)
    ohd = P("ohd", [128, TOTS, 128], BF16)
    gpool = P("gpool", [NCH, 128, G], BF16)
    w2 = P("w2", [L, D, D], BF16)
    b2 = P("b2", [L, D, 1])
    w1n = P("w1n", [2, D, D], BF16)
    b1n = P("b1n", [2, D, 1])
    bng = P("bng", [L, D, 1])
    bnb = P("bnb", [L, D, 1])
    mw1 = P("mw1", [D, 64])
    mb1 = P("mb1", [64, 1])
    mw2 = P("mw2", [64, 32])
    mb2 = P("mb2", [32, 1])
    mw3 = P("mw3", [32, OUT])
    mb3 = P("mb3", [OUT, 1])
    out = nc.declare_dram_parameter("out", [OUT, G], F32, isOutput=True)

    yag_in = nc.dram_tensor("yag_in", [128, SHP], BF16)
    tblag = nc.dram_tensor("tblag", [NCORES, 128, SHP], BF16, addr_space="Shared")
    ccin = nc.dram_tensor("ccin", [128, 2], F32)
    ccout = nc.dram_tensor("ccout", [128, 2], F32, addr_space="Shared")
    plin = nc.dram_tensor("plin", [128, G], F32)
    plout = nc.dram_tensor("plout", [128, G], F32, addr_space="Shared")

    RG = [list(range(NCORES))]
    AF = mybir.ActivationFunctionType
    X = mybir.AxisListType.X

    with tile.TileContext(nc) as tc:
        with tc.tile_pool(name="const", bufs=1) as cp, \
             tc.tile_pool(name="big", bufs=1) as bp, \
             tc.tile_pool(name="sb", bufs=3) as sb, \
             tc.tile_pool(name="st", bufs=2) as stp, \
             tc.tile_pool(name="pp", bufs=3, space="PSUM") as pp, \
             tc.tile_pool(name="ppb", bufs=2, space="PSUM") as ppb, \
             tc.tile_pool(name="ppz", bufs=2, space="PSUM") as ppz, \
             tc.tile_pool(name="pp1", bufs=1, space="PSUM") as pp1:

            ident = cp.tile([128, 128], BF16, tag="ident")
            make_identity(nc, ident[:])
            epst = cp.tile([128, 1], F32, tag="epst")
            nc.vector.memset(epst[:], EPS)

            w2_t, w1n_t, b2_t, b1n_t, bng_t, bnb_t = [], [], [], [], [], []
            for l in range(L):
                a = cp.tile([D, D], BF16, tag=f"w2_{l}")
                nc.sync.dma_start(out=a[:], in_=w2[l])
                w2_t.append(a)
                for tlist, srcp, nm in ((b2_t, b2, "b2"), (bng_t, bng, "bng"),
                                        (bnb_t, bnb, "bnb")):
                    a = cp.tile([D, 1], F32, tag=f"{nm}_{l}")
                    nc.sync.dma_start(out=a[:], in_=srcp[l])
                    tlist.append(a)
            for l in range(2):
                a = cp.tile([D, D], BF16, tag=f"w1n_{l}")
                nc.sync.dma_start(out=a[:], in_=w1n[l])
                w1n_t.append(a)
                a = cp.tile([D, 1], F32, tag=f"b1n_{l}")
                nc.sync.dma_start(out=a[:], in_=b1n[l])
                b1n_t.append(a)
            mw1_t = cp.tile([D, 64], F32, tag="mw1")
            nc.sync.dma_start(out=mw1_t[:], in_=mw1[:])
            mw2_t = cp.tile([64, 32], F32, tag="mw2")
            nc.sync.dma_start(out=mw2_t[:], in_=mw2[:])
            mw3_t = cp.tile([32, OUT], F32, tag="mw3")
            nc.sync.dma_start(out=mw3_t[:], in_=mw3[:])
            mb1_t = cp.tile([64, 1], F32, tag="mb1")
            nc.sync.dma_start(out=mb1_t[:], in_=mb1[:])
            mb2_t = cp.tile([32, 1], F32, tag="mb2")
            nc.sync.dma_start(out=mb2_t[:], in_=mb2[:])
            mb3_t = cp.tile([OUT, 1], F32, tag="mb3")
            nc.sync.dma_start(out=mb3_t[:], in_=mb3[:])

            tbl = bp.tile([128, TR], BF16, tag="tbl")
            nc.sync.dma_start(out=tbl[:], in_=x0tab[:])
            yown = bp.tile([128, SHP], F32, tag="yown")
            nc.sync.dma_start(out=yown[:], in_=y0[:])
            z1 = bp.tile([128, SHP], BF16, tag="z1")
            zfin = bp.tile([128, SHP], BF16, tag="zfin")
            yag = bp.tile([128, SHP], BF16, tag="yag")
            stats_s = bp.tile([128, 64], F32, tag="ss")
            stats_q = bp.tile([128, 64], F32, tag="sq")
            pooled_ps = pp1.tile([128, G], F32, tag="pooled")

            for l in range(L):
                # ---- pass A: gather + aggregate + BN stats ----
                for k in range(NCH):
                    Sk = S[k]
                    off = int(offs[k])
                    ks = slice(k * 128, (k + 1) * 128)
                    ncols = LASTC if k == NCH - 1 else 128
                    xi = sb.tile([128, SMAX * 8], I16, tag="xi")
                    nc.sync.dma_start(out=xi[:, :Sk * 8],
                                      in_=idxd[:, off * 8:(off + Sk) * 8])
                    ohT = sb.tile([128, SMAX, 128], BF16, tag="oh")
                    nc.sync.dma_start(out=ohT[:, :Sk, :],
                                      in_=ohd[:, off:off + Sk, :])
                    gf = sb.tile([128, SMAX * 128, 2], BF16, tag="gf")
                    nc.gpsimd.ap_gather(gf[:, :Sk * 128, :], tbl[:],
                                        xi[:, :Sk * 8], channels=128,
                                        num_elems=TR // 2, d=2,
                                        num_idxs=Sk * 128)
                    gem = sb.tile([128, SMAX, 128], BF16, tag="gem")
                    for t in range(Sk):
                        par = 0 if t < SE[k] else 1
                        ptr = pp.tile([128, 128], BF16, tag="ptr")
                        nc.tensor.transpose(out=ptr[:],
                                            in_=gf[:, t * 128:(t + 1) * 128, par],
                                            identity=ident[:])
                        if t % 2 == 0:
                            nc.scalar.activation(gem[:, t, :], ptr[:], AF.Copy)
                        else:
                            nc.vector.tensor_copy(out=gem[:, t, :], in_=ptr[:])
                    pz = ppz.tile([128, 128], F32, tag="pz")
                    for t in range(Sk):
                        nc.tensor.matmul(pz[:], gem[:, t, :], ohT[:, t, :],
                                         start=(t == 0), stop=(t == Sk - 1))
                    nc.vector.tensor_tensor(out=z1[:, ks], in0=pz[:],
                                            in1=yown[:, ks],
                                            op=mybir.AluOpType.add)
                    zsl = z1[:, k * 128: k * 128 + ncols]
                    nc.vector.reduce_sum(out=stats_s[:, k:k + 1], in_=zsl, axis=X)
                    sqd = sb.tile([128, 128], BF16, tag="sqd")
                    nc.scalar.activation(sqd[:, :ncols], zsl, AF.Square,
                                         accum_out=stats_q[:, k:k + 1])
                # ---- BN stats: AllReduce + scale/shift ----
                s_tot = stp.tile([128, 1], F32, tag="s_tot")
                nc.vector.reduce_sum(out=s_tot[:], in_=stats_s[:, :NCH], axis=X)
                q_tot = stp.tile([128, 1], F32, tag="q_tot")
                nc.vector.reduce_sum(out=q_tot[:], in_=stats_q[:, :NCH], axis=X)
                cc = stp.tile([128, 2], F32, tag="cc")
                nc.vector.tensor_copy(out=cc[:, 0:1], in_=s_tot[:])
                nc.vector.tensor_copy(out=cc[:, 1:2], in_=q_tot[:])
                nc.sync.dma_start(out=ccin[:], in_=cc[:])
                nc.gpsimd.collective_compute(
                    "AllReduce", mybir.AluOpType.add, replica_groups=RG,
                    ins=[ccin[:]], outs=[ccout[:]])
                stg = stp.tile([128, 2], F32, tag="stg")
                nc.sync.dma_start(out=stg[:], in_=ccout[:])
                mean = stp.tile([128, 1], F32, tag="mean")
                nc.vector.tensor_scalar_mul(out=mean[:], in0=stg[:, 0:1],
                                            scalar1=1.0 / N)
                ex2 = stp.tile([128, 1], F32, tag="ex2")
                nc.vector.tensor_scalar_mul(out=ex2[:], in0=stg[:, 1:2],
                                            scalar1=1.0 / N)
                msq = stp.tile([128, 1], F32, tag="msq")
                nc.vector.tensor_tensor(out=msq[:], in0=mean[:], in1=mean[:],
                                        op=mybir.AluOpType.mult)
                var = stp.tile([128, 1], F32, tag="var")
                nc.vector.tensor_tensor(out=var[:], in0=ex2[:], in1=msq[:],
                                        op=mybir.AluOpType.subtract)
                sqv = stp.tile([128, 1], F32, tag="sqv")
                nc.scalar.activation(sqv[:], var[:], AF.Sqrt, bias=epst[:])
                rstd = stp.tile([128, 1], F32, tag="rstd")
                nc.vector.reciprocal(out=rstd[:], in_=sqv[:])
                scale = stp.tile([128, 1], F32, tag="scale")
                nc.vector.tensor_tensor(out=scale[:], in0=bng_t[l][:], in1=rstd[:],
                                        op=mybir.AluOpType.mult)
                mscl = stp.tile([128, 1], F32, tag="mscl")
                nc.vector.tensor_tensor(out=mscl[:], in0=mean[:], in1=scale[:],
                                        op=mybir.AluOpType.mult)
                shift = stp.tile([128, 1], F32, tag="shift")
                nc.vector.tensor_tensor(out=shift[:], in0=bnb_t[l][:], in1=mscl[:],
                                        op=mybir.AluOpType.subtract)
                # ---- pass B: BN+relu, W2 (+relu), W1_next fold ----
                for k in range(NCH):
                    ks = slice(k * 128, (k + 1) * 128)
                    z1n = sb.tile([128, 128], BF16, tag="z1n")
                    nc.scalar.activation(z1n[:], z1[:, ks], AF.Relu,
                                         bias=shift[:], scale=scale[:])
                    psm = ppb.tile([128, 128], F32, tag="pb")
                    nc.tensor.matmul(psm[:], w2_t[l][:], z1n[:], start=True,
                                     stop=True)
                    if l < L - 1:
                        z2t = sb.tile([128, 128], BF16, tag="z2t")
                        nc.scalar.activation(z2t[:], psm[:], AF.Relu,
                                             bias=b2_t[l][:])
                        psy = ppb.tile([128, 128], F32, tag="pb")
                        nc.tensor.matmul(psy[:], w1n_t[l][:], z2t[:], start=True,
                                         stop=True)
                        nc.vector.tensor_scalar_add(out=yown[:, ks], in0=psy[:],
                                                    scalar1=b1n_t[l][:])
                        nc.scalar.activation(yag[:, ks], psy[:], AF.Copy)
                    else:
                        nc.vector.tensor_scalar_add(out=zfin[:, ks], in0=psm[:],
                                                    scalar1=b2_t[l][:])
                if l < L - 1:
                    nc.sync.dma_start(out=yag_in[:], in_=yag[:])
                    nc.gpsimd.collective_compute(
                        "AllGather", mybir.AluOpType.bypass, replica_groups=RG,
                        ins=[yag_in[:]], outs=[tblag[:]])
                    for c in range(NCORES):
                        nc.sync.dma_start(out=tbl[:, c * SHP:(c + 1) * SHP],
                                          in_=tblag[c])

            # ---- pooling + AllReduce + readout MLP ----
            for k in range(NCH):
                ks = slice(k * 128, (k + 1) * 128)
                ptr = pp.tile([128, 128], BF16, tag="ptr")
                nc.tensor.transpose(out=ptr[:], in_=zfin[:, ks], identity=ident[:])
                nmt = sb.tile([128, 128], BF16, tag="nmt")
                nc.scalar.activation(nmt[:], ptr[:], AF.Copy)
                gp = sb.tile([128, G], BF16, tag="gp")
                nc.sync.dma_start(out=gp[:], in_=gpool[k])
                nc.tensor.matmul(pooled_ps[:], nmt[:], gp[:],
                                 start=(k == 0), stop=(k == NCH - 1))
            plt = stp.tile([128, G], F32, tag="plt")
            nc.vector.tensor_copy(out=plt[:], in_=pooled_ps[:])
            nc.sync.dma_start(out=plin[:], in_=plt[:])
            nc.gpsimd.collective_compute(
                "AllReduce", mybir.AluOpType.add, replica_groups=RG,
                ins=[plin[:]], outs=[plout[:]])
            pl = stp.tile([128, G], F32, tag="pl")
            nc.sync.dma_start(out=pl[:], in_=plout[:])
            ps_rt = ppb.tile([128, 128], F32, tag="pb")
            ps_r = ps_rt[:64, :G]
            nc.tensor.matmul(ps_r, mw1_t[:], pl[:], start=True, stop=True)
            r1 = stp.tile([64, G], F32, tag="r1")
            nc.scalar.activation(r1[:], ps_r, AF.Relu, bias=mb1_t[:])
            ps_r2t = ppb.tile([128, 128], F32, tag="pb")
            ps_r2 = ps_r2t[:32, :G]
            nc.tensor.matmul(ps_r2, mw2_t[:], r1[:], start=True, stop=True)
            r2 = stp.tile([32, G], F32, tag="r2")
            nc.scalar.activation(r2[:], ps_r2, AF.Relu, bias=mb2_t[:])
            ps_r3t = ppb.tile([128, 128], F32, tag="pb")
            ps_r3 = ps_r3t[:OUT, :G]
            nc.tensor.matmul(ps_r3, mw3_t[:], r2[:], start=True, stop=True)
            ot = stp.tile([OUT, G], F32, tag="ot")
            nc.vector.tensor_scalar_add(out=ot[:], in0=ps_r3, scalar1=mb3_t[:])
            nc.sync.dma_start(out=out[:], in_=ot[:])
    nc.compile()
    return nc


def _prep(x, edge_index, edge_attr, batch,
          lin1_w, lin1_b, bn_g, bn_b, lin2_w, lin2_b,
          mlp_w1, mlp_b1, mlp_w2, mlp_b2, mlp_w3, mlp_b3):
    x = np.asarray(x, np.float32)
    ei = np.asarray(edge_index).astype(np.int64)
    batch = np.asarray(batch).astype(np.int64)
    lin1_w = np.asarray(lin1_w, np.float32)
    lin1_b = np.asarray(lin1_b, np.float32)
    src, dst = ei[0], ei[1]

    srccol = ((src // SH) * SHP + (src % SH)).astype(np.int64)
    parity = (srccol & 1).astype(np.int64)
    core = dst // SH
    chunk = (dst % SH) // 128
    dcol = (dst % SH) % 128
    # sort edges by (core, chunk, parity): even-src edges first within a chunk
    key = ((core * NCH + chunk) * 2 + parity).astype(np.int64)
    order = np.argsort(key, kind="stable")
    srccol_s, dcol_s = srccol[order], dcol[order]
    pcounts = np.bincount(key[order], minlength=NCORES * NCH * 2)
    pcounts = pcounts.reshape(NCORES, NCH, 2)
    SE = np.maximum(1, -(-pcounts[:, :, 0].max(axis=0) // 128)).astype(int)
    SO = np.maximum(1, -(-pcounts[:, :, 1].max(axis=0) // 128)).astype(int)
    S = SE + SO
    offs = np.concatenate([[0], np.cumsum(S)]).astype(int)
    TOTS = int(S.sum())
    starts = np.zeros(NCORES * NCH * 2 + 1, np.int64)
    np.cumsum(pcounts.reshape(-1), out=starts[1:])

    idxd_a = np.zeros((NCORES, 128, TOTS * 8), np.int16)
    ohd_a = np.zeros((NCORES, 128, TOTS, 128), NBF)
    for c in range(NCORES):
        for k in range(NCH):
            off = int(offs[k])
            for par in (0, 1):
                kk = (c * NCH + k) * 2 + par
                s, e = starts[kk], starts[kk + 1]
                sc, dc = srccol_s[s:e] >> 1, dcol_s[s:e]
                n = len(sc)
                if n == 0:
                    continue
                base = 0 if par == 0 else int(SE[k]) * 128
                i = np.arange(n) + base
                ohd_a[c, i % 128, off + i // 128, dc] = 1.0
                for g in range(8):
                    idxd_a[c, g * 16 + (i % 16), off * 8 + i // 16] = sc

    # layer-0 table: y0 = x @ W1_0 (f32 on host), f-major, bf16
    y0full = x @ lin1_w[0]
    x0tab_a = np.zeros((128, TR), NBF)
    y0_a = np.zeros((NCORES, 128, SHP), np.float32)
    for c in range(NCORES):
        ys = y0full[c * SH:(c + 1) * SH]
        x0tab_a[:, c * SHP: c * SHP + SH] = ys.T.astype(NBF)
        y0_a[c, :, :SH] = ys.T + lin1_b[0][:, None]

    cnts = np.bincount(batch, minlength=G).astype(np.float32)
    inv = 1.0 / np.maximum(cnts, 1.0)
    gpool_a = np.zeros((NCORES, NCH, 128, G), NBF)
    for c in range(NCORES):
        b = batch[c * SH:(c + 1) * SH]
        p = np.arange(len(b))
        gpool_a[c, p // 128, p % 128, b] = inv[b]

    common = {
        "x0tab": x0tab_a,
        "w2": np.asarray(lin2_w, NBF),
        "b2": np.asarray(lin2_b, np.float32).reshape(L, D, 1),
        "w1n": np.asarray(lin1_w[1:], NBF),
        "b1n": np.asarray(lin1_b[1:], np.float32).reshape(2, D, 1),
        "bng": np.asarray(bn_g, np.float32).reshape(L, D, 1),
        "bnb": np.asarray(bn_b, np.float32).reshape(L, D, 1),
        "mw1": np.asarray(mlp_w1, np.float32),
        "mb1": np.asarray(mlp_b1, np.float32).reshape(64, 1),
        "mw2": np.asarray(mlp_w2, np.float32),
        "mb2": np.asarray(mlp_b2, np.float32).reshape(32, 1),
        "mw3": np.asarray(mlp_w3, np.float32),
        "mb3": np.asarray(mlp_b3, np.float32).reshape(OUT, 1),
    }
    in_maps = []
    for c in range(NCORES):
        m = dict(common)
        m["y0"] = y0_a[c]
        m["idxd"] = idxd_a[c]
        m["ohd"] = ohd_a[c]
        m["gpool"] = gpool_a[c]
        in_maps.append(m)
    return tuple((int(a), int(b)) for a, b in zip(SE, SO)), in_maps


def kernel(**inputs):
    S, in_maps = _prep(**inputs)
    if S not in _cache:
        _cache[S] = _build(S)
    r = run_bass_kernel_spmd(_cache[S], in_maps, list(range(NCORES)))
    return np.ascontiguousarray(np.asarray(r.results[0]["out"]).T.astype(np.float32))
